# revision 23
# baseline (speedup 1.0000x reference)
"""Trainium2 Bass kernel for nn_AlarmworkRNN.

Key insight: in the reference, each row i of the [max_seq_len, num_hidden]
carried states evolves independently (row i of z1_new depends only on row i
of z1, z2, x_seq), and the output taps only row -1 (= 2047). So the entire
computation reduces to a sequential chain of 1024-dim vector-matrix products
on that single row:

    z1_{t+1} = tanh(c1[t] + (z1_t + z2_t) @ W_rec1)        (256 steps)
    z2 updates only on even steps:  z2' = tanh(c2[t] + z2 @ W_rec2)  (128 upd)
    out[t]   = tanh(z1_{t+1} @ W_out + b_out)

with c1 = xr @ W_in1 + b_in1 (xr = x[:, -1, :]) batched up front, the
z2-dependent term z2_t @ W_rec1 batched into G after the z2 chain, and the
output matmul batched at the end.

The per-step critical path is a [1,1024]x[1024,1024] matvec on the PE array
(z as the 1-column stationary operand per 128-chunk, W streamed as the
moving operand in fp32r at 1 col/cycle). State vectors live partition-major
in SBUF "stack" tiles [128, 2, T+1] so they can feed the next step's
stationary loads; each step's tanh output row is scattered back across
partitions by a small SBUF->SBUF DMA, pipelined so it hides under the next
step's streaming.

All 8 cores run the identical program (the sequential chain cannot be
usefully split: per-step cross-core sync costs ~5us >> the 4us step); the
output is taken from core 0.
"""

import numpy as np

import concourse.bass as bass
import concourse.mybir as mybir
from concourse import bacc
from concourse.tile import TileContext

F32 = mybir.dt.float32
DT = mybir.dt.float32r  # matmul operand dtype: fp32 bits, 1 col/cycle stream
Tanh = mybir.ActivationFunctionType.Tanh

D = 256      # num_data (scan steps)
I = 256      # num_inputs
H = 1024     # num_hidden
O = 256      # num_outputs
HC = H // 128  # 8 chunks of the hidden dim


def build_nc(n1=D, n2=D // 2, n_cores=8):
    """Build the Bass program. n1 = z1 steps, n2 = z2 updates (n1//2)."""
    nc = bacc.Bacc("TRN2", target_bir_lowering=False, debug=False)

    # ---- kernel I/O ----
    xrT = nc.dram_tensor("xrT", [I, D], DT, kind="ExternalInput")  # x[:,-1,:].T
    W_in1 = nc.dram_tensor("W_in1", [I, H], DT, kind="ExternalInput")
    b_in1 = nc.dram_tensor("b_in1", [1, H], DT, kind="ExternalInput")
    W_rec1 = nc.dram_tensor("W_rec1", [H, H], DT, kind="ExternalInput")
    W_in2 = nc.dram_tensor("W_in2", [I, H], DT, kind="ExternalInput")
    b_in2 = nc.dram_tensor("b_in2", [1, H], DT, kind="ExternalInput")
    W_rec2 = nc.dram_tensor("W_rec2", [H, H], DT, kind="ExternalInput")
    W_out = nc.dram_tensor("W_out", [H, O], DT, kind="ExternalInput")
    b_out = nc.dram_tensor("b_out", [1, O], DT, kind="ExternalInput")
    out = nc.dram_tensor("out", [D, O], F32, kind="ExternalOutput")

    with TileContext(nc) as tc:
        with (
            tc.tile_pool(name="static", bufs=1) as sp,
            tc.tile_pool(name="dram", bufs=1, space="DRAM") as dp,
            tc.tile_pool(name="arow", bufs=4) as apool,
            tc.tile_pool(name="rows", bufs=2) as rpool,
        ):
            # ---- static SBUF tiles ----
            W1sb = sp.tile([128, HC, H], DT, tag="W1sb")
            W2sb = sp.tile([128, HC, H], DT, tag="W2sb")
            Wosb = sp.tile([128, HC, O], DT, tag="Wosb")
            Win1sb = sp.tile([128, 2, H], DT, tag="Win1sb")
            Win2sb = sp.tile([128, 2, H], DT, tag="Win2sb")
            xrsb = sp.tile([128, 2, D], DT, tag="xrsb")
            b1sb = sp.tile([1, H], DT, tag="b1sb")
            b2sb = sp.tile([1, H], DT, tag="b2sb")
            bosb = sp.tile([1, O], DT, tag="bosb")
            ones = sp.tile([1, 128], DT, tag="ones")
            C1sb = [sp.tile([128, H], DT, tag=f"C1sb{i}", name=f"C1sb{i}") for i in range(2)]
            C2sb = sp.tile([128, H], DT, tag="C2sb")
            z1s = [sp.tile([128, n1 + 1, 2], DT, tag=f"z1s{q}", name=f"z1s{q}") for q in range(4)]
            z2s = [sp.tile([128, n2 + 1, 2], DT, tag=f"z2s{q}", name=f"z2s{q}") for q in range(4)]
            Gsb0 = sp.tile([128, H], DT, tag="Gsb0")
            Gsb1 = sp.tile([1, H], DT, tag="Gsb1")

            C1d = dp.tile([n1, H], DT)
            Gd_d = dp.tile([n2 + 1, H], DT)
            A2d = dp.tile([n2, H], DT)

            # ---- load weights (z2-phase tensors first) ----
            nc.sync.dma_start(out=xrsb[:], in_=xrT.ap().rearrange("(k p) t -> p k t", p=128))
            nc.sync.dma_start(out=Win2sb[:], in_=W_in2.ap().rearrange("(k p) n -> p k n", p=128))
            nc.sync.dma_start(out=b2sb[:], in_=b_in2.ap())
            nc.gpsimd.dma_start(out=W2sb[:], in_=W_rec2.ap().rearrange("(k p) n -> p k n", p=128))
            nc.gpsimd.dma_start(out=W1sb[:], in_=W_rec1.ap().rearrange("(k p) n -> p k n", p=128))
            nc.gpsimd.dma_start(out=Win1sb[:], in_=W_in1.ap().rearrange("(k p) n -> p k n", p=128))
            nc.gpsimd.dma_start(out=b1sb[:], in_=b_in1.ap())
            nc.gpsimd.dma_start(out=Wosb[:], in_=W_out.ap().rearrange("(k p) o -> p k o", p=128))
            nc.gpsimd.dma_start(out=bosb[:], in_=b_out.ap())
            nc.gpsimd.memset(ones[:].bitcast(F32), 1.0)
            for q in range(4):
                nc.gpsimd.memset(z1s[q][:, 0:1, :].bitcast(F32), 0.0)
                nc.gpsimd.memset(z2s[q][:, 0:1, :].bitcast(F32), 0.0)

            # ---- preamble: C2 = xr_even @ W_in2 + b2 -> A2d; C1 stays in SBUF
            with tc.tile_pool(name="ppre", bufs=2, space="PSUM") as ppre:
                for nh in range(2):
                    pc = ppre.tile([128, 512], F32, tag="pc2")
                    for k in range(2):
                        nc.tensor.matmul(pc[:, :], lhsT=xrsb[:, k, 0:D:2],
                                         rhs=Win2sb[:, k, nh * 512:(nh + 1) * 512],
                                         start=(k == 0), stop=False)
                    nc.tensor.matmul(pc[:, :], lhsT=ones[0:1, 0:128],
                                     rhs=b2sb[0:1, nh * 512:(nh + 1) * 512],
                                     start=False, stop=True)
                    nc.vector.tensor_copy(out=C2sb[:, nh * 512:(nh + 1) * 512], in_=pc[:, :])
                nc.sync.dma_start(out=A2d[0:n2, :], in_=C2sb[0:n2, :])
                for th in range(2):
                    for nh in range(2):
                        pc = ppre.tile([128, 512], F32, tag="pc1")
                        for k in range(2):
                            nc.tensor.matmul(pc[:, :], lhsT=xrsb[:, k, th * 128:(th + 1) * 128],
                                             rhs=Win1sb[:, k, nh * 512:(nh + 1) * 512],
                                             start=(k == 0), stop=False)
                        nc.tensor.matmul(pc[:, :], lhsT=ones[0:1, 0:128],
                                         rhs=b1sb[0:1, nh * 512:(nh + 1) * 512],
                                         start=False, stop=True)
                        nc.vector.tensor_copy(out=C1sb[th][:, nh * 512:(nh + 1) * 512], in_=pc[:, :])
                for th in range((n1 + 127) // 128):
                    tlo = th * 128
                    tcnt = min(128, n1 - tlo)
                    nc.sync.dma_start(out=C1d[tlo:tlo + tcnt, :], in_=C1sb[th][0:tcnt, :])
                ppre.release()

            # ---- the sequential matvec chain (shared by z2 and z1 phases) ----
            # The Tile scheduler is free to reorder same-engine instructions,
            # which destroys the stall-hiding stagger; chain PE and ACT
            # instructions into the intended total order explicitly.
            from concourse.tile_rust import add_dep_helper
            prev_inst = {}

            def seq(eng, op):
                if eng in prev_inst:
                    add_dep_helper(op.ins, prev_inst[eng].ins, sync=False,
                                   reason="enforce schedule order")
                prev_inst[eng] = op
                return op

            # per-step PE order: group q covers output cols [256q, 256q+256)
            # and produces state chunks (2q, 2q+1). Stops (the trailing fold)
            # are staggered so each group's tanh+scatter hides under later
            # groups' streaming, and the last-produced chunks (6,7) are
            # consumed latest in the next step.
            ORDER = ([(0, 0), (0, 1), (1, 0), (1, 1),
                      (0, 2), (0, 3), (1, 2), (1, 3),
                      (0, 4), (0, 5), (1, 4), (1, 5)]
                     + [(2, k) for k in range(6)]
                     + [(0, 6), (0, 7), (0, -1)]
                     + [(1, 6), (1, 7), (1, -1)]
                     + [(2, 6), (2, 7), (2, -1)]
                     + [(3, k) for k in range(8)] + [(3, -1)])

            def chain(nsteps, Wsb, stacks, Ad, Gd=None):
                # per-step fold row a_t: for z2, a_t = Ad[t]; for z1,
                # a_t = Ad[t] + Gd[(t+1)//2] (added on the idle DVE).
                abufs, cbufs, gbufs = {}, {}, {}

                # prefetches go on the gpsimd (SWDGE) ring: HWDGE rings are
                # FIFO per engine, so a prefetch stalled on a buffer-reuse
                # fence would head-of-line block the latency-critical
                # scatters behind it.
                def prefetch(t):
                    if t >= nsteps:
                        return
                    if Gd is None:
                        abufs[t] = apool.tile([1, H], DT, tag="aa", name="aa")
                        nc.sync.dma_start(out=abufs[t][0:1, :], in_=Ad[t:t + 1, :])
                    else:
                        cbufs[t] = apool.tile([1, H], DT, tag="ac", name="ac")
                        nc.sync.dma_start(out=cbufs[t][0:1, :], in_=Ad[t:t + 1, :])
                        m = (t + 1) // 2
                        if m not in gbufs:
                            gbufs[m] = apool.tile([1, H], DT, tag="ag", name="ag")
                            nc.sync.dma_start(out=gbufs[m][0:1, :], in_=Gd[m:m + 1, :])

                def makea(t):
                    if t >= nsteps or Gd is None:
                        return
                    abufs[t] = apool.tile([1, H], DT, tag="aa", name="aa")
                    m = (t + 1) // 2
                    nc.vector.tensor_add(out=abufs[t][0:1, :],
                                         in0=cbufs.pop(t)[0:1, :], in1=gbufs[m][0:1, :])
                    if m > 0 and (t + 2) // 2 != m:
                        gbufs.pop(m - 1, None)

                prefetch(0)
                prefetch(1)
                prefetch(2)
                makea(0)
                for t in range(nsteps):
                    prefetch(t + 3)
                    makea(t + 1)
                    a = abufs.pop(t)
                    ps = [pch.tile([128, 256], F32, tag=f"u{q}", name=f"u{q}") for q in range(4)]
                    started = set()
                    for q, k in ORDER:
                        if k < 0:
                            # fold a_t into the psum group, last (stop=True)
                            seq("pe", nc.tensor.matmul(
                                ps[q][0:1, :], lhsT=ones[0:1, 0:1],
                                rhs=a[0:1, q * 256:(q + 1) * 256],
                                start=False, stop=True))
                            row = rpool.tile([1, 256], DT, tag=f"r{q}", name=f"r{q}")
                            seq("act", nc.scalar.activation(row[0:1, :], ps[q][0:1, :], Tanh))
                            eng = nc.sync if q < 2 else nc.scalar
                            sc = eng.dma_start(
                                out=stacks[q][:, t + 1, :],
                                in_=row[0:1, :].rearrange("a (p c) -> a p c", c=2))
                            if q >= 2:
                                seq("act", sc)
                        else:
                            seq("pe", nc.tensor.matmul(
                                ps[q][0:1, :],
                                lhsT=stacks[k // 2][:, t:t + 1, k % 2],
                                rhs=Wsb[:, k, q * 256:(q + 1) * 256],
                                start=(q not in started), stop=False))
                            started.add(q)

            # ---- phase 1: z2 chain ----
            with tc.tile_pool(name="pch2", bufs=2, space="PSUM") as pch:
                chain(n2, W2sb, z2s, A2d, "a2", "r2")

            # ---- phase 2: G = Z2stack @ W_rec1 ; A1 = C1 + G_dup ----
            with tc.tile_pool(name="pg", bufs=2, space="PSUM") as pg:
                for mb, mcnt, gdst in ((0, min(128, n2 + 1), Gsb0), (128, n2 + 1 - 128, Gsb1)):
                    if mcnt <= 0:
                        continue
                    for nh in range(2):
                        pgt = pg.tile([128, 512], F32, tag="pg")
                        for k in range(HC):
                            nc.tensor.matmul(pgt[0:mcnt, :],
                                             lhsT=z2s[k // 2][:, mb:mb + mcnt, k % 2],
                                             rhs=W1sb[:, k, nh * 512:(nh + 1) * 512],
                                             start=(k == 0), stop=(k == HC - 1))
                        nc.vector.tensor_copy(out=gdst[0:mcnt, nh * 512:(nh + 1) * 512],
                                              in_=pgt[0:mcnt, :])
                # store G contiguously; the z1 chain adds C1d[t] + Gd[(t+1)//2]
                # per step on the idle vector engine.
                nc.sync.dma_start(out=Gd_d[0:min(128, n2 + 1), :],
                                  in_=Gsb0[0:min(128, n2 + 1), :])
                if n2 + 1 > 128:
                    nc.sync.dma_start(out=Gd_d[128:n2 + 1, :], in_=Gsb1[0:1, :])

            # ---- phase 3: z1 chain ----
            with tc.tile_pool(name="pch1", bufs=2, space="PSUM") as pch:
                chain(n1, W1sb, z1s, C1d, "a1", "r1", Gd=Gd_d)

            # ---- phase 4: OUT = tanh(Z1stack[:,1:].T @ W_out + b_out) ----
            with (
                tc.tile_pool(name="pfin", bufs=2, space="PSUM") as pf,
                tc.tile_pool(name="ofin", bufs=2) as opool,
            ):
                tbs = [(0, min(128, n1))]
                if n1 > 128:
                    tbs.append((128, n1 - 128))
                for tb, tcnt in tbs:
                    po = pf.tile([128, O], F32, tag="po")
                    for k in range(HC):
                        nc.tensor.matmul(po[0:tcnt, :],
                                         lhsT=z1s[k // 2][:, 1 + tb:1 + tb + tcnt, k % 2],
                                         rhs=Wosb[:, k, :],
                                         start=(k == 0), stop=False)
                    nc.tensor.matmul(po[0:tcnt, :], lhsT=ones[0:1, 0:tcnt],
                                     rhs=bosb[0:1, :], start=False, stop=True)
                    orow = opool.tile([128, O], F32, tag="orow")
                    nc.scalar.activation(orow[0:tcnt, :], po[0:tcnt, :], Tanh)
                    nc.sync.dma_start(out=out.ap()[tb:tb + tcnt, :], in_=orow[0:tcnt, :])

    nc.compile()
    return nc


# Within each 256-wide output group q, permute the hidden columns so that
# column l' = 2*p + c holds hidden unit n = q*256 + c*128 + p. The per-step
# tanh row then lands in (partition, chunk-pair) order, making the row ->
# stack scatter DMA a contiguous 8B-per-partition transfer. C1/C2/G and the
# fold rows inherit the same order, so everything stays consistent; the
# stacks and the output matmul see canonical hidden indexing.
_PERM = np.empty(H, np.int64)
for _q in range(4):
    for _c in range(2):
        for _p in range(128):
            _PERM[_q * 256 + 2 * _p + _c] = _q * 256 + _c * 128 + _p


def make_in_map(x, W_in1, b_in1, W_rec1, W_in2, b_in2, W_rec2, W_out, b_out):
    xr = np.ascontiguousarray(np.asarray(x)[:, -1, :].T, dtype=np.float32)
    f = lambda a: np.asarray(a, dtype=np.float32)
    pc = lambda w: np.ascontiguousarray(f(w)[:, _PERM])
    return {
        "xrT": xr,
        "W_in1": pc(W_in1), "b_in1": np.ascontiguousarray(f(b_in1).reshape(1, H)[:, _PERM]),
        "W_rec1": pc(W_rec1),
        "W_in2": pc(W_in2), "b_in2": np.ascontiguousarray(f(b_in2).reshape(1, H)[:, _PERM]),
        "W_rec2": pc(W_rec2),
        "W_out": np.ascontiguousarray(f(W_out)), "b_out": np.ascontiguousarray(f(b_out).reshape(1, O)),
    }


_cached = {}


def kernel(**inputs) -> np.ndarray:
    from concourse.bass_utils import run_bass_kernel_spmd

    if "nc" not in _cached:
        _cached["nc"] = build_nc()
    nc = _cached["nc"]
    in_map = make_in_map(**inputs)
    n_cores = 8
    res = run_bass_kernel_spmd(nc, [dict(in_map) for _ in range(n_cores)],
                               core_ids=list(range(n_cores)))
    return np.asarray(res.results[0]["out"], dtype=np.float32)


if __name__ == "__main__":
    import reference as R

    inputs = {k: np.asarray(v) for k, v in R.setup_inputs().items()}
    got = kernel(**inputs)
    print("out", got.shape, got.dtype)


# revision 24
# speedup vs baseline: 1.0120x; 1.0120x over previous
"""Trainium2 Bass kernel for nn_AlarmworkRNN.

Key insight: in the reference, each row i of the [max_seq_len, num_hidden]
carried states evolves independently (row i of z1_new depends only on row i
of z1, z2, x_seq), and the output taps only row -1 (= 2047). So the entire
computation reduces to a sequential chain of 1024-dim vector-matrix products
on that single row:

    z1_{t+1} = tanh(c1[t] + (z1_t + z2_t) @ W_rec1)        (256 steps)
    z2 updates only on even steps:  z2' = tanh(c2[t] + z2 @ W_rec2)  (128 upd)
    out[t]   = tanh(z1_{t+1} @ W_out + b_out)

with c1 = xr @ W_in1 + b_in1 (xr = x[:, -1, :]) batched up front, the
z2-dependent term z2_t @ W_rec1 batched into G after the z2 chain, and the
output matmul batched at the end.

The per-step critical path is a [1,1024]x[1024,1024] matvec on the PE array
(z as the 1-column stationary operand per 128-chunk, W streamed as the
moving operand in fp32r at 1 col/cycle). State vectors live partition-major
in SBUF "stack" tiles [128, 2, T+1] so they can feed the next step's
stationary loads; each step's tanh output row is scattered back across
partitions by a small SBUF->SBUF DMA, pipelined so it hides under the next
step's streaming.

All 8 cores run the identical program (the sequential chain cannot be
usefully split: per-step cross-core sync costs ~5us >> the 4us step); the
output is taken from core 0.
"""

import numpy as np

import concourse.bass as bass
import concourse.mybir as mybir
from concourse import bacc
from concourse.tile import TileContext

F32 = mybir.dt.float32
DT = mybir.dt.float32r  # matmul operand dtype: fp32 bits, 1 col/cycle stream
Tanh = mybir.ActivationFunctionType.Tanh

D = 256      # num_data (scan steps)
I = 256      # num_inputs
H = 1024     # num_hidden
O = 256      # num_outputs
HC = H // 128  # 8 chunks of the hidden dim


def build_nc(n1=D, n2=D // 2, n_cores=8, pipelined=True, gs=64):
    """Build the Bass program. n1 = z1 steps, n2 = z2 updates (n1//2)."""
    nc = bacc.Bacc("TRN2", target_bir_lowering=False, debug=False,
                   num_devices=(8 if pipelined else None))

    # ---- kernel I/O ----
    xrT = nc.dram_tensor("xrT", [I, D], DT, kind="ExternalInput")  # x[:,-1,:].T
    W_in1 = nc.dram_tensor("W_in1", [I, H], DT, kind="ExternalInput")
    b_in1 = nc.dram_tensor("b_in1", [1, H], DT, kind="ExternalInput")
    W_rec1 = nc.dram_tensor("W_rec1", [H, H], DT, kind="ExternalInput")
    W_in2 = nc.dram_tensor("W_in2", [I, H], DT, kind="ExternalInput")
    b_in2 = nc.dram_tensor("b_in2", [1, H], DT, kind="ExternalInput")
    W_rec2 = nc.dram_tensor("W_rec2", [H, H], DT, kind="ExternalInput")
    W_out = nc.dram_tensor("W_out", [H, O], DT, kind="ExternalInput")
    b_out = nc.dram_tensor("b_out", [1, O], DT, kind="ExternalInput")
    out = nc.dram_tensor("out", [D, O], F32, kind="ExternalOutput")

    with TileContext(nc) as tc:
        with (
            tc.tile_pool(name="static", bufs=1) as sp,
            tc.tile_pool(name="dram", bufs=1, space="DRAM") as dp,
        ):
            # ---- static SBUF tiles ----
            W1sb = sp.tile([128, HC, H], DT, tag="W1sb")
            W2sb = sp.tile([128, HC, H], DT, tag="W2sb")
            Wosb = sp.tile([128, HC, O], DT, tag="Wosb")
            Win1sb = sp.tile([128, 2, H], DT, tag="Win1sb")
            Win2sb = sp.tile([128, 2, H], DT, tag="Win2sb")
            xrsb = sp.tile([128, 2, D], DT, tag="xrsb")
            b1sb = sp.tile([1, H], DT, tag="b1sb")
            b2sb = sp.tile([1, H], DT, tag="b2sb")
            bosb = sp.tile([1, O], DT, tag="bosb")
            ones = sp.tile([1, 128], DT, tag="ones")
            C1sb = [sp.tile([128, H], DT, tag=f"C1sb{i}", name=f"C1sb{i}") for i in range(2)]
            C2sb = sp.tile([128, H], DT, tag="C2sb")
            z1s = [sp.tile([128, n1 + 1, 2], DT, tag=f"z1s{q}", name=f"z1s{q}") for q in range(4)]
            z2s = [sp.tile([128, n2 + 1, 2], DT, tag=f"z2s{q}", name=f"z2s{q}") for q in range(4)]
            Gsb0 = sp.tile([128, H], DT, tag="Gsb0")
            Gsb1 = sp.tile([1, H], DT, tag="Gsb1")

            C1d = dp.tile([n1, H], DT)
            A2d = dp.tile([n2, H], DT)
            # z2 slice boundaries for the core pipeline: first slice small
            # so core 0's initial wait is short, sized so the second slice
            # arrives before core 0 consumes the first.
            if n2 > gs:
                SL = [(0, gs), (gs, n2)]
            else:
                SL = [(0, n2)]
            NS = len(SL)
            if pipelined:
                Gd0 = dp.tile([1, H], DT, name="Gd0")
                Gbounce = [dp.tile([b - a, H], DT, name=f"Gb{i}", tag=f"Gb{i}")
                           for i, (a, b) in enumerate(SL)]
                Gout = [dp.tile([b - a, H], DT, name=f"Go{i}", tag=f"Go{i}",
                                addr_space="Shared") for i, (a, b) in enumerate(SL)]
            else:
                Gd_d = dp.tile([n2 + 1, H], DT)

            # ---- load weights (z2-phase tensors first) ----
            nc.sync.dma_start(out=xrsb[:], in_=xrT.ap().rearrange("(k p) t -> p k t", p=128))
            nc.sync.dma_start(out=Win2sb[:], in_=W_in2.ap().rearrange("(k p) n -> p k n", p=128))
            nc.sync.dma_start(out=b2sb[:], in_=b_in2.ap())
            nc.gpsimd.dma_start(out=W2sb[:], in_=W_rec2.ap().rearrange("(k p) n -> p k n", p=128))
            nc.gpsimd.dma_start(out=W1sb[:], in_=W_rec1.ap().rearrange("(k p) n -> p k n", p=128))
            nc.gpsimd.dma_start(out=Win1sb[:], in_=W_in1.ap().rearrange("(k p) n -> p k n", p=128))
            nc.gpsimd.dma_start(out=b1sb[:], in_=b_in1.ap())
            nc.gpsimd.dma_start(out=Wosb[:], in_=W_out.ap().rearrange("(k p) o -> p k o", p=128))
            nc.gpsimd.dma_start(out=bosb[:], in_=b_out.ap())
            nc.gpsimd.memset(ones[:].bitcast(F32), 1.0)
            for q in range(4):
                nc.gpsimd.memset(z1s[q][:, 0:1, :].bitcast(F32), 0.0)
                nc.gpsimd.memset(z2s[q][:, 0:1, :].bitcast(F32), 0.0)

            # ---- preamble: C2 = xr_even @ W_in2 + b2 -> A2d; C1 stays in SBUF
            with tc.tile_pool(name="ppre", bufs=2, space="PSUM") as ppre:
                for nh in range(2):
                    pc = ppre.tile([128, 512], F32, tag="pc2")
                    for k in range(2):
                        nc.tensor.matmul(pc[:, :], lhsT=xrsb[:, k, 0:D:2],
                                         rhs=Win2sb[:, k, nh * 512:(nh + 1) * 512],
                                         start=(k == 0), stop=False)
                    nc.tensor.matmul(pc[:, :], lhsT=ones[0:1, 0:128],
                                     rhs=b2sb[0:1, nh * 512:(nh + 1) * 512],
                                     start=False, stop=True)
                    nc.vector.tensor_copy(out=C2sb[:, nh * 512:(nh + 1) * 512], in_=pc[:, :])
                nc.sync.dma_start(out=A2d[0:n2, :], in_=C2sb[0:n2, :])
                for th in range(2):
                    for nh in range(2):
                        pc = ppre.tile([128, 512], F32, tag="pc1")
                        for k in range(2):
                            nc.tensor.matmul(pc[:, :], lhsT=xrsb[:, k, th * 128:(th + 1) * 128],
                                             rhs=Win1sb[:, k, nh * 512:(nh + 1) * 512],
                                             start=(k == 0), stop=False)
                        nc.tensor.matmul(pc[:, :], lhsT=ones[0:1, 0:128],
                                         rhs=b1sb[0:1, nh * 512:(nh + 1) * 512],
                                         start=False, stop=True)
                        nc.vector.tensor_copy(out=C1sb[th][:, nh * 512:(nh + 1) * 512], in_=pc[:, :])
                for th in range((n1 + 127) // 128):
                    tlo = th * 128
                    tcnt = min(128, n1 - tlo)
                    nc.sync.dma_start(out=C1d[tlo:tlo + tcnt, :], in_=C1sb[th][0:tcnt, :])
                ppre.release()

            apool = tc.alloc_tile_pool(name="arow", bufs=4)
            rpool = tc.alloc_tile_pool(name="rows", bufs=2)

            # ---- the sequential matvec chain (shared by z2 and z1 phases) ----
            # The Tile scheduler is free to reorder same-engine instructions,
            # which destroys the stall-hiding stagger; chain PE and ACT
            # instructions into the intended total order explicitly.
            from concourse.tile_rust import add_dep_helper
            prev_inst = {}

            def seq(eng, op):
                if eng in prev_inst:
                    add_dep_helper(op.ins, prev_inst[eng].ins, sync=False,
                                   reason="enforce schedule order")
                prev_inst[eng] = op
                return op

            # per-step PE order: group q covers output cols [256q, 256q+256)
            # and produces state chunks (2q, 2q+1). Stops (the trailing fold)
            # are staggered so each group's tanh+scatter hides under later
            # groups' streaming, and the last-produced chunks (6,7) are
            # consumed latest in the next step.
            ORDER = ([(0, 0), (0, 1), (1, 0), (1, 1),
                      (0, 2), (0, 3), (1, 2), (1, 3),
                      (0, 4), (0, 5), (1, 4), (1, 5)]
                     + [(2, k) for k in range(6)]
                     + [(0, 6), (0, 7), (0, -1)]
                     + [(1, 6), (1, 7), (1, -1)]
                     + [(2, 6), (2, 7), (2, -1)]
                     + [(3, k) for k in range(8)] + [(3, -1)])

            def make_chain(nsteps, Wsb, stacks, Ad, sfx, Gd=None):
                # per-step fold row a_t: for z2, a_t = Ad[t]; for z1,
                # a_t = Ad[t] + Gd(m) with m=(t+1)//2 (added on the idle DVE).
                abufs, cbufs, gbufs = {}, {}, {}

                # prefetches go on the gpsimd (SWDGE) ring: HWDGE rings are
                # FIFO per engine, so a prefetch stalled on a buffer-reuse
                # fence would head-of-line block the latency-critical
                # scatters behind it.
                def prefetch(t):
                    if t >= nsteps:
                        return
                    if Gd is None:
                        abufs[t] = apool.tile([1, H], DT, tag="aa" + sfx, name="aa" + sfx)
                        nc.sync.dma_start(out=abufs[t][0:1, :], in_=Ad[t:t + 1, :])
                    else:
                        cbufs[t] = apool.tile([1, H], DT, tag="ac" + sfx, name="ac" + sfx)
                        nc.sync.dma_start(out=cbufs[t][0:1, :], in_=Ad[t:t + 1, :])
                        m = (t + 1) // 2
                        if m not in gbufs:
                            gbufs[m] = apool.tile([1, H], DT, tag="ag" + sfx, name="ag" + sfx)
                            nc.sync.dma_start(out=gbufs[m][0:1, :], in_=Gd(m))

                def makea(t):
                    if t >= nsteps or Gd is None:
                        return
                    abufs[t] = apool.tile([1, H], DT, tag="aa" + sfx, name="aa" + sfx)
                    m = (t + 1) // 2
                    nc.vector.tensor_add(out=abufs[t][0:1, :],
                                         in0=cbufs.pop(t)[0:1, :], in1=gbufs[m][0:1, :])
                    if m > 0 and (t + 2) // 2 != m:
                        gbufs.pop(m - 1, None)

                def emit(t0, t1):
                  if t0 == 0:
                    prefetch(0)
                    prefetch(1)
                    prefetch(2)
                    makea(0)
                  for t in range(t0, t1):
                    prefetch(t + 3)
                    makea(t + 1)
                    a = abufs.pop(t)
                    ps = [pch.tile([128, 256], F32, tag=f"u{q}", name=f"u{q}") for q in range(4)]
                    started = set()
                    for q, k in ORDER:
                        if k < 0:
                            # fold a_t into the psum group, last (stop=True)
                            seq("pe", nc.tensor.matmul(
                                ps[q][0:1, :], lhsT=ones[0:1, 0:1],
                                rhs=a[0:1, q * 256:(q + 1) * 256],
                                start=False, stop=True))
                            row = rpool.tile([1, 256], DT, tag=f"r{q}{sfx}", name=f"r{q}{sfx}")
                            seq("act", nc.scalar.activation(row[0:1, :], ps[q][0:1, :], Tanh))
                            eng = nc.sync if q < 2 else nc.scalar
                            sc = eng.dma_start(
                                out=stacks[q][:, t + 1, :],
                                in_=row[0:1, :].rearrange("a (p c) -> a p c", c=2))
                            if q >= 2:
                                seq("act", sc)
                        else:
                            seq("pe", nc.tensor.matmul(
                                ps[q][0:1, :],
                                lhsT=stacks[k // 2][:, t:t + 1, k % 2],
                                rhs=Wsb[:, k, q * 256:(q + 1) * 256],
                                start=(q not in started), stop=False))
                            started.add(q)

            # ---- phase 1: z2 chain ----
            with tc.tile_pool(name="pch2", bufs=2, space="PSUM") as pch:
                chain(n2, W2sb, z2s, A2d, "a2", "r2")

            # ---- phase 2: G = Z2stack @ W_rec1 ; A1 = C1 + G_dup ----
            with tc.tile_pool(name="pg", bufs=2, space="PSUM") as pg:
                for mb, mcnt, gdst in ((0, min(128, n2 + 1), Gsb0), (128, n2 + 1 - 128, Gsb1)):
                    if mcnt <= 0:
                        continue
                    for nh in range(2):
                        pgt = pg.tile([128, 512], F32, tag="pg")
                        for k in range(HC):
                            nc.tensor.matmul(pgt[0:mcnt, :],
                                             lhsT=z2s[k // 2][:, mb:mb + mcnt, k % 2],
                                             rhs=W1sb[:, k, nh * 512:(nh + 1) * 512],
                                             start=(k == 0), stop=(k == HC - 1))
                        nc.vector.tensor_copy(out=gdst[0:mcnt, nh * 512:(nh + 1) * 512],
                                              in_=pgt[0:mcnt, :])
                # store G contiguously; the z1 chain adds C1d[t] + Gd[(t+1)//2]
                # per step on the idle vector engine.
                nc.sync.dma_start(out=Gd_d[0:min(128, n2 + 1), :],
                                  in_=Gsb0[0:min(128, n2 + 1), :])
                if n2 + 1 > 128:
                    nc.sync.dma_start(out=Gd_d[128:n2 + 1, :], in_=Gsb1[0:1, :])

            # ---- phase 3: z1 chain ----
            with tc.tile_pool(name="pch1", bufs=2, space="PSUM") as pch:
                chain(n1, W1sb, z1s, C1d, "a1", "r1", Gd=Gd_d)

            # ---- phase 4: OUT = tanh(Z1stack[:,1:].T @ W_out + b_out) ----
            with (
                tc.tile_pool(name="pfin", bufs=2, space="PSUM") as pf,
                tc.tile_pool(name="ofin", bufs=2) as opool,
            ):
                tbs = [(0, min(128, n1))]
                if n1 > 128:
                    tbs.append((128, n1 - 128))
                for tb, tcnt in tbs:
                    po = pf.tile([128, O], F32, tag="po")
                    for k in range(HC):
                        nc.tensor.matmul(po[0:tcnt, :],
                                         lhsT=z1s[k // 2][:, 1 + tb:1 + tb + tcnt, k % 2],
                                         rhs=Wosb[:, k, :],
                                         start=(k == 0), stop=False)
                    nc.tensor.matmul(po[0:tcnt, :], lhsT=ones[0:1, 0:tcnt],
                                     rhs=bosb[0:1, :], start=False, stop=True)
                    orow = opool.tile([128, O], F32, tag="orow")
                    nc.scalar.activation(orow[0:tcnt, :], po[0:tcnt, :], Tanh)
                    nc.sync.dma_start(out=out.ap()[tb:tb + tcnt, :], in_=orow[0:tcnt, :])

    nc.compile()
    return nc


# Within each 256-wide output group q, permute the hidden columns so that
# column l' = 2*p + c holds hidden unit n = q*256 + c*128 + p. The per-step
# tanh row then lands in (partition, chunk-pair) order, making the row ->
# stack scatter DMA a contiguous 8B-per-partition transfer. C1/C2/G and the
# fold rows inherit the same order, so everything stays consistent; the
# stacks and the output matmul see canonical hidden indexing.
_PERM = np.empty(H, np.int64)
for _q in range(4):
    for _c in range(2):
        for _p in range(128):
            _PERM[_q * 256 + 2 * _p + _c] = _q * 256 + _c * 128 + _p


def make_in_map(x, W_in1, b_in1, W_rec1, W_in2, b_in2, W_rec2, W_out, b_out):
    xr = np.ascontiguousarray(np.asarray(x)[:, -1, :].T, dtype=np.float32)
    f = lambda a: np.asarray(a, dtype=np.float32)
    pc = lambda w: np.ascontiguousarray(f(w)[:, _PERM])
    return {
        "xrT": xr,
        "W_in1": pc(W_in1), "b_in1": np.ascontiguousarray(f(b_in1).reshape(1, H)[:, _PERM]),
        "W_rec1": pc(W_rec1),
        "W_in2": pc(W_in2), "b_in2": np.ascontiguousarray(f(b_in2).reshape(1, H)[:, _PERM]),
        "W_rec2": pc(W_rec2),
        "W_out": np.ascontiguousarray(f(W_out)), "b_out": np.ascontiguousarray(f(b_out).reshape(1, O)),
    }


_cached = {}


def kernel(**inputs) -> np.ndarray:
    from concourse.bass_utils import run_bass_kernel_spmd

    if "nc" not in _cached:
        _cached["nc"] = build_nc()
    nc = _cached["nc"]
    in_map = make_in_map(**inputs)
    n_cores = 8
    res = run_bass_kernel_spmd(nc, [dict(in_map) for _ in range(n_cores)],
                               core_ids=list(range(n_cores)))
    return np.asarray(res.results[0]["out"], dtype=np.float32)


if __name__ == "__main__":
    import reference as R

    inputs = {k: np.asarray(v) for k, v in R.setup_inputs().items()}
    got = kernel(**inputs)
    print("out", got.shape, got.dtype)
